# revision 13
# baseline (speedup 1.0000x reference)
"""AttentionPairBias Trainium2 Bass kernel.

Problem: B=1, N=1024, C_A=768, C_Z=128, H=16, CH=48.
Sharding: query-token sharding across 8 cores (core i owns q in
[128*i, 128*(i+1))). No collectives: each core recomputes LN(a) and the
full K/V projections (cheap vs the z pair-bias path), and processes only
its own 128-query slice of z ([128, 1024, 128] = 64 MB/core), logits,
softmax, AV, gating and output projection rows.

Key layout choices:
 - Transposed Q/K ("head dim on partitions"), heads padded 48->64 so each
   128-partition chunk holds exactly 2 heads.
 - logitsT[k, q] computed directly (k on partitions) so softmax runs
   without any attn transpose: exp via ACT (no max subtraction -- logits
   are O(1) by construction), denominator via a ones-column matmul
   against the same exp weights, normalization folded into the output.
 - Pair bias: LN(z) @ Wb is decomposed algebraically so the matmul runs
   on raw z tiles: bpair = r*(z @ (w.*Wb)) - r*m*S1 + S2 with per-row
   stats (m, r) from bn_stats on the natural-layout tile, and the matmul
   on the PE-transposed tile.
"""

import numpy as np

B, N = 1, 1024
D = 768            # C_A
CZ = 128
H = 16
CH = 48
CHP = 64           # padded head dim (2 heads per 128-partition chunk)
QB = 128           # queries per core
NCORES = 8
DC = D // 128      # 6 d-chunks
NT = N // 128      # 8 token chunks
SCALE = float(1.0 / np.sqrt(CH))
EPS = 1e-5

_CACHE = {}


def _build_nc(dbg=False):
    from contextlib import ExitStack

    import concourse.mybir as mybir
    import concourse.tile as tile
    from concourse import bacc
    import concourse.bass as bass
    from concourse.masks import make_identity

    f32 = mybir.dt.float32
    bf16 = mybir.dt.bfloat16
    AF = mybir.ActivationFunctionType
    OP = mybir.AluOpType
    AX = mybir.AxisListType

    nc = bacc.Bacc("TRN2", target_bir_lowering=False, debug=False)

    # ---- DRAM tensors ----
    a_d = nc.dram_tensor("a", [N, D], f32, kind="ExternalInput").ap()
    aq_d = nc.dram_tensor("aq", [QB, D], f32, kind="ExternalInput").ap()
    zq_d = nc.dram_tensor("zq", [QB * N, CZ], f32, kind="ExternalInput").ap()
    mask_d = nc.dram_tensor("mask", [N], f32, kind="ExternalInput").ap()
    lnaw_d = nc.dram_tensor("ln_a_w", [D], f32, kind="ExternalInput").ap()
    lnab_d = nc.dram_tensor("ln_a_b", [D], f32, kind="ExternalInput").ap()
    lnzw_d = nc.dram_tensor("ln_z_w", [CZ], f32, kind="ExternalInput").ap()
    lnzb_d = nc.dram_tensor("ln_z_b", [CZ], f32, kind="ExternalInput").ap()
    wq_d = nc.dram_tensor("Wq", [D, D], f32, kind="ExternalInput").ap()
    bq_d = nc.dram_tensor("bq", [D], f32, kind="ExternalInput").ap()
    wk_d = nc.dram_tensor("Wk", [D, D], f32, kind="ExternalInput").ap()
    wv_d = nc.dram_tensor("Wv", [D, D], f32, kind="ExternalInput").ap()
    wb_d = nc.dram_tensor("Wb", [CZ, H], f32, kind="ExternalInput").ap()
    wg_d = nc.dram_tensor("Wg", [D, D], f32, kind="ExternalInput").ap()
    wo_d = nc.dram_tensor("Wo", [D, D], f32, kind="ExternalInput").ap()
    out_d = nc.dram_tensor("out", [QB, D], f32, kind="ExternalOutput").ap()
    if dbg:
        dbg_an = nc.dram_tensor("dbg_an", [QB, D], f32, kind="ExternalOutput").ap()
        dbg_kt = nc.dram_tensor("dbg_kt", [128, N], f32, kind="ExternalOutput").ap()
        dbg_qt = nc.dram_tensor("dbg_qt", [128, QB], f32, kind="ExternalOutput").ap()
        dbg_v = nc.dram_tensor("dbg_v", [128, H * (CH + 1)], f32, kind="ExternalOutput").ap()
        dbg_g = nc.dram_tensor("dbg_g", [QB, D], f32, kind="ExternalOutput").ap()
        dbg_bias = nc.dram_tensor("dbg_bias", [128, 4 * QB * H], bf16, kind="ExternalOutput").ap()
        dbg_osb = nc.dram_tensor("dbg_osb", [QB, D], f32, kind="ExternalOutput").ap()
        dbg_ex0 = nc.dram_tensor("dbg_ex0", [128, 128], f32, kind="ExternalOutput").ap()
        dbg_ex1 = nc.dram_tensor("dbg_ex1", [128, 128], f32, kind="ExternalOutput").ap()
        dbg_po = nc.dram_tensor("dbg_po", [128, CH + 1], f32, kind="ExternalOutput").ap()

    def bcast_ap(src, p):
        # DMA access pattern that replicates a 1-D (or row) DRAM region
        # across p partitions.
        return bass.AP(tensor=src.tensor, offset=src.offset, ap=[[0, p], *src.ap])

    with tile.TileContext(nc) as tc, ExitStack() as ctx:
        singles = ctx.enter_context(tc.tile_pool(name="singles", bufs=1))
        persist = ctx.enter_context(tc.tile_pool(name="persist", bufs=1))
        small = ctx.enter_context(tc.tile_pool(name="small", bufs=4))

        # ---- constants / preprococessed weights ----
        ident = singles.tile([128, 128], f32)
        make_identity(nc, ident[:])

        eps_t = singles.tile([128, 1], f32)
        nc.vector.memset(eps_t[:], EPS)

        lnaw_b = singles.tile([128, D], f32)
        nc.gpsimd.dma_start(out=lnaw_b[:], in_=bcast_ap(lnaw_d, 128))
        lnab_b = singles.tile([128, D], f32)
        nc.gpsimd.dma_start(out=lnab_b[:], in_=bcast_ap(lnab_d, 128))

        # Wb path: Wp = [ln_z_w .* Wb | ones] (17 cols), S1/S2 broadcast rows
        lnzw_t = singles.tile([128, 1], f32)
        nc.gpsimd.dma_start(out=lnzw_t[:], in_=lnzw_d[:, None])
        lnzb_t = singles.tile([128, 1], f32)
        nc.gpsimd.dma_start(out=lnzb_t[:], in_=lnzb_d[:, None])
        wb_t = singles.tile([128, H], f32)
        nc.sync.dma_start(out=wb_t[:], in_=wb_d[:, :])
        wp = singles.tile([128, H + 1], f32)
        nc.vector.tensor_scalar_mul(out=wp[:, 0:H], in0=wb_t[:], scalar1=lnzw_t[:])
        nc.vector.memset(wp[:, H : H + 1], 1.0)
        wb2 = singles.tile([128, H], f32)
        nc.vector.tensor_scalar_mul(out=wb2[:], in0=wb_t[:], scalar1=lnzb_t[:])

        ones_t = singles.tile([128, 128], f32)
        nc.vector.memset(ones_t[:], 1.0)

        with tc.tile_pool(name="spsum", bufs=2, space="PSUM") as spsum:
            s1_p = spsum.tile([128, H], f32)
            nc.tensor.matmul(s1_p[:], ones_t[:], wp[:, 0:H], start=True, stop=True)
            s1_b = singles.tile([128, H], f32)
            nc.vector.tensor_copy(out=s1_b[:], in_=s1_p[:])
            s2_p = spsum.tile([128, H], f32)
            nc.tensor.matmul(s2_p[:], ones_t[:], wb2[:], start=True, stop=True)
            s2_b = singles.tile([128, H], f32)
            nc.vector.tensor_copy(out=s2_b[:], in_=s2_p[:])

        # mask bias per k-chunk: mb = (mask - 1) * 1e9  -> [128, 1] per chunk
        mb = []
        for t in range(NT):
            m_t = singles.tile([128, 1], f32, tag=f"mb{t}")
            nc.gpsimd.dma_start(out=m_t[:], in_=mask_d[t * 128 : (t + 1) * 128, None])
            nc.vector.tensor_scalar(
                out=m_t[:], in0=m_t[:], scalar1=-1.0, scalar2=1.0e9,
                op0=OP.add, op1=OP.mult,
            )
            mb.append(m_t)

        # bq padded+scaled per hc'-chunk
        bqs = []
        for j in range(NT):
            b_t = singles.tile([128, 1], f32, tag=f"bqs{j}")
            nc.vector.memset(b_t[:], 0.0)
            nc.gpsimd.dma_start(
                out=b_t[0:CH, :], in_=bq_d[96 * j : 96 * j + 48, None]
            )
            nc.gpsimd.dma_start(
                out=b_t[CHP : CHP + CH, :], in_=bq_d[96 * j + 48 : 96 * j + 96, None]
            )
            nc.vector.tensor_scalar_mul(out=b_t[:], in0=b_t[:], scalar1=SCALE)
            bqs.append(b_t)

        # ---- LayerNorm(a) for all tokens, then transpose to anT ----
        def ln_rows(x_tile, p=128):
            # in-place LayerNorm over free dim D (768 = 3 x 256 bn groups)
            stats = small.tile([p, 3, 6], f32, tag="lnstats")
            xg = x_tile[:].rearrange("p (g d) -> p g d", g=3)
            for g in range(3):
                nc.vector.bn_stats(out=stats[:, g, :], in_=xg[:, g, :])
            mv = small.tile([p, 2], f32, tag="lnmv")
            nc.vector.bn_aggr(out=mv[:], in_=stats[:])
            nc.scalar.activation(
                out=mv[:, 1:2], in_=mv[:, 1:2], func=AF.Sqrt, bias=eps_t[:]
            )
            nc.vector.reciprocal(out=mv[:, 1:2], in_=mv[:, 1:2])
            nc.vector.tensor_scalar(
                out=x_tile[:], in0=x_tile[:], scalar1=mv[:, 0:1], scalar2=mv[:, 1:2],
                op0=OP.subtract, op1=OP.mult,
            )
            nc.vector.tensor_mul(out=x_tile[:], in0=x_tile[:], in1=lnaw_b[:p, :])
            nc.vector.tensor_add(out=x_tile[:], in0=x_tile[:], in1=lnab_b[:p, :])

        anT = [persist.tile([128, N], f32, tag=f"anT{j}", name=f"anT{j}") for j in range(DC)]
        anTq = [persist.tile([128, QB], f32, tag=f"anTq{j}", name=f"anTq{j}") for j in range(DC)]

        apool = ctx.enter_context(tc.tile_pool(name="apool", bufs=3))
        with tc.tile_pool(name="trpsum", bufs=4, space="PSUM") as trpsum:
            for t in range(NT):
                a_t = apool.tile([128, D], f32, tag="a_t")
                nc.sync.dma_start(out=a_t[:], in_=a_d[t * 128 : (t + 1) * 128, :])
                ln_rows(a_t)
                if dbg and t == 0:
                    nc.sync.dma_start(out=dbg_an[:, :], in_=a_t[:])
                for j in range(DC):
                    ps = trpsum.tile([128, 128], f32, tag="tr")
                    nc.tensor.transpose(
                        ps[:], a_t[:, j * 128 : (j + 1) * 128], ident[:]
                    )
                    nc.any.tensor_copy(
                        out=anT[j][:, t * 128 : (t + 1) * 128], in_=ps[:]
                    )
            # own-query block LN + transpose
            aq_t = apool.tile([128, D], f32, tag="a_t")
            nc.sync.dma_start(out=aq_t[:], in_=aq_d[:, :])
            ln_rows(aq_t)
            for j in range(DC):
                ps = trpsum.tile([128, 128], f32, tag="tr")
                nc.tensor.transpose(ps[:], aq_t[:, j * 128 : (j + 1) * 128], ident[:])
                nc.any.tensor_copy(out=anTq[j][:], in_=ps[:])

        # ---- projections ----
        # padded weight tile loader: [128 d, 128 hc'] with cols 0:48 and
        # 64:112 from the two heads of chunk j
        wpool = ctx.enter_context(tc.tile_pool(name="wpool", bufs=6))

        def w_pad_tile(w_dram, d, j, tag):
            wt = wpool.tile([128, 128], f32, tag=tag)
            nc.vector.memset(wt[:], 0.0)
            nc.sync.dma_start(
                out=wt[:, 0:CH],
                in_=w_dram[d * 128 : (d + 1) * 128, 96 * j : 96 * j + 48],
            )
            nc.sync.dma_start(
                out=wt[:, CHP : CHP + CH],
                in_=w_dram[d * 128 : (d + 1) * 128, 96 * j + 48 : 96 * j + 96],
            )
            return wt

        KT = [persist.tile([128, N], f32, tag=f"KT{j}", name=f"KT{j}") for j in range(NT)]
        QT = [persist.tile([128, QB], f32, tag=f"QT{j}", name=f"QT{j}") for j in range(NT)]
        V = [persist.tile([128, H * (CH + 1)], f32, tag=f"V{t}", name=f"V{t}") for t in range(NT)]
        g_sb = persist.tile([128, D], f32, tag="g_sb")

        with tc.tile_pool(name="prpsum", bufs=2, space="PSUM") as prpsum:
            # K^T padded: [hc' chunk, tok]
            for j in range(NT):
                for half in range(2):
                    ps = prpsum.tile([128, 512], f32, tag="prk")
                    for d in range(DC):
                        wt = w_pad_tile(wk_d, d, j, tag="wk")
                        nc.tensor.matmul(
                            ps[:],
                            wt[:],
                            anT[d][:, half * 512 : (half + 1) * 512],
                            start=(d == 0),
                            stop=(d == DC - 1),
                        )
                    nc.any.tensor_copy(
                        out=KT[j][:, half * 512 : (half + 1) * 512], in_=ps[:]
                    )
            # Q^T padded (own queries), scaled by 1/sqrt(CH), +bq
            for j in range(NT):
                ps = prpsum.tile([128, 128], f32, tag="prq")
                for d in range(DC):
                    wt = w_pad_tile(wq_d, d, j, tag="wq")
                    nc.tensor.matmul(
                        ps[:], wt[:], anTq[d][:], start=(d == 0), stop=(d == DC - 1)
                    )
                nc.scalar.activation(
                    out=QT[j][:], in_=ps[:], func=AF.Identity,
                    bias=bqs[j][:], scale=SCALE,
                )
            # V natural: [tok chunk, 768] + ones col
            for t in range(NT):
                ps = prpsum.tile([128, D], f32, tag="prv")
                for d in range(DC):
                    wv_t = wpool.tile([128, D], f32, tag="wv")
                    nc.sync.dma_start(
                        out=wv_t[:], in_=wv_d[d * 128 : (d + 1) * 128, :]
                    )
                    nc.tensor.matmul(
                        ps[:, 0:512], anT[d][:, t * 128 : (t + 1) * 128],
                        wv_t[:, 0:512], start=(d == 0), stop=(d == DC - 1),
                    )
                    nc.tensor.matmul(
                        ps[:, 512:D], anT[d][:, t * 128 : (t + 1) * 128],
                        wv_t[:, 512:D], start=(d == 0), stop=(d == DC - 1),
                    )
                for h in range(H):
                    nc.any.tensor_copy(
                        out=V[t][:, (CH + 1) * h : (CH + 1) * h + CH],
                        in_=ps[:, CH * h : CH * h + CH],
                    )
                nc.vector.memset(
                    V[t][:].rearrange("p (h c) -> p h c", h=H)[:, :, CH : CH + 1],
                    1.0,
                )
            # gate (own queries, natural layout) + sigmoid
            ps = prpsum.tile([128, D], f32, tag="prv")
            for d in range(DC):
                wg_t = wpool.tile([128, D], f32, tag="wv")
                nc.sync.dma_start(out=wg_t[:], in_=wg_d[d * 128 : (d + 1) * 128, :])
                nc.tensor.matmul(
                    ps[:, 0:512], anTq[d][:], wg_t[:, 0:512],
                    start=(d == 0), stop=(d == DC - 1),
                )
                nc.tensor.matmul(
                    ps[:, 512:D], anTq[d][:], wg_t[:, 512:D],
                    start=(d == 0), stop=(d == DC - 1),
                )
            nc.scalar.activation(out=g_sb[:], in_=ps[:], func=AF.Sigmoid)

        # ---- z pair-bias path + attention ----
        # bias buffers: bf16 [128k, j(4), q(128), h(16)] per kc-group
        bias_g = [
            persist.tile([128, 4, QB, H], bf16, tag=f"biasg{g}", name=f"biasg{g}") for g in range(2)
        ]

        zpool = ctx.enter_context(tc.tile_pool(name="zpool", bufs=3))
        ztpool = ctx.enter_context(tc.tile_pool(name="ztpool", bufs=6))
        fxpool = ctx.enter_context(tc.tile_pool(name="fxpool", bufs=4))

        zq_r = zq_d.rearrange("(q g j p) c -> q g j p c", q=QB, g=2, j=4)

        psum_o = ctx.enter_context(tc.tile_pool(name="psum_o", bufs=2, space="PSUM"))
        zctx = ExitStack()
        psum_z = zctx.enter_context(tc.tile_pool(name="psum_z", bufs=2, space="PSUM"))
        psum_b = zctx.enter_context(tc.tile_pool(name="psum_b", bufs=2, space="PSUM"))
        psum_l = zctx.enter_context(tc.tile_pool(name="psum_l", bufs=2, space="PSUM"))

        o_sb = persist.tile([128, D], f32, tag="o_sb")

        def z_slab(q, g):
            # load [128 k-rows, 4 k-subchunks, 128 c] natural slab
            zs = zpool.tile([128, 4, CZ], f32, tag="zs")
            src = bass.AP(
                tensor=zq_r.tensor,
                offset=zq_r.offset
                + q * zq_r.ap[0][0]
                + g * zq_r.ap[1][0],
                ap=[zq_r.ap[3], zq_r.ap[2], zq_r.ap[4]],
            )
            nc.sync.dma_start(out=zs[:], in_=src)

            # per-row stats via bn_stats (mean, var)
            mv4 = fxpool.tile([128, 4, 2], f32, tag="mv4")
            for j in range(4):
                st = fxpool.tile([128, 6], f32, tag="zst")
                nc.vector.bn_stats(out=st[:], in_=zs[:, j, :])
                nc.vector.bn_aggr(out=mv4[:, j, :], in_=st[:])
            # rstd in place of var
            nc.scalar.activation(
                out=mv4[:].rearrange("p j s -> p (j s)")[:, 1::2].rearrange(
                    "p j -> p j"
                ),
                in_=mv4[:].rearrange("p j s -> p (j s)")[:, 1::2],
                func=AF.Sqrt,
                bias=eps_t[:],
            )
            rec_in = mv4[:].rearrange("p j s -> p (j s)")[:, 1::2]
            nc.vector.reciprocal(out=rec_in, in_=rec_in)

            # transpose 4 tiles, matmul z^T @ Wp -> [k, 17]
            pb = psum_b.tile([128, 4, H + 1], f32, tag="pb")
            for j in range(4):
                pt = psum_z.tile([128, 128], f32, tag="zt")
                nc.tensor.transpose(pt[:], zs[:, j, :], ident[:])
                zt_sb = ztpool.tile([128, 128], f32, tag="zt_sb")
                nc.any.tensor_copy(out=zt_sb[:], in_=pt[:])
                nc.tensor.matmul(pb[:, j, :], zt_sb[:], wp[:], start=True, stop=True)

            # fixup: bias = r*(raw - mean*S1) + S2  (batched over j, h)
            mean_b = bass.AP(
                tensor=mv4.tensor, offset=mv4[:].offset,
                ap=[mv4[:].ap[0], [2, 4], [0, H]],
            )
            r_b = bass.AP(
                tensor=mv4.tensor, offset=mv4[:].offset + 1,
                ap=[mv4[:].ap[0], [2, 4], [0, H]],
            )
            s1_4 = bass.AP(
                tensor=s1_b.tensor, offset=s1_b[:].offset,
                ap=[s1_b[:].ap[0], [0, 4], [1, H]],
            )
            s2_4 = bass.AP(
                tensor=s2_b.tensor, offset=s2_b[:].offset,
                ap=[s2_b[:].ap[0], [0, 4], [1, H]],
            )
            t1 = fxpool.tile([128, 4, H], f32, tag="fx1")
            nc.vector.tensor_tensor(out=t1[:], in0=mean_b, in1=s1_4, op=OP.mult)
            nc.vector.tensor_tensor(
                out=t1[:], in0=pb[:, :, 0:H], in1=t1[:], op=OP.subtract
            )
            nc.vector.tensor_tensor(out=t1[:], in0=t1[:], in1=r_b, op=OP.mult)
            nc.vector.tensor_tensor(
                out=bias_g[g][:, :, q, :], in0=t1[:], in1=s2_4, op=OP.add
            )

        expool = ctx.enter_context(tc.tile_pool(name="expool", bufs=4))

        def attn_head(h):
            # one contiguous PSUM accumulation group per head (own bank)
            cj, off = divmod(h, 2)
            po_t = psum_o.tile([128, CH + 1], f32, tag="po_t")
            for kc in range(NT):
                g, jj = divmod(kc, 4)
                pl = psum_l.tile([128, 128], f32, tag="pl")
                nc.tensor.matmul(
                    pl[:],
                    KT[cj][CHP * off : CHP * off + CH, kc * 128 : (kc + 1) * 128],
                    QT[cj][CHP * off : CHP * off + CH, :],
                    start=True, stop=True,
                )
                ex = expool.tile([128, 128], f32, tag="ex")
                nc.vector.tensor_tensor(
                    out=ex[:], in0=pl[:], in1=bias_g[g][:, jj, :, h], op=OP.add
                )
                nc.scalar.activation(
                    out=ex[:], in_=ex[:], func=AF.Exp, bias=mb[kc][:]
                )
                if dbg and kc == 0 and h in (0, 1):
                    nc.sync.dma_start(
                        out=(dbg_ex0 if h == 0 else dbg_ex1)[:, :], in_=ex[:]
                    )
                nc.tensor.matmul(
                    po_t[:],
                    ex[:],
                    V[kc][:, (CH + 1) * h : (CH + 1) * (h + 1)],
                    start=(kc == 0), stop=(kc == NT - 1),
                )
            rs = small.tile([128, 1], f32, tag="rs")
            if dbg and h == 0:
                po_f = small.tile([128, CH + 1], f32, tag="po_f")
                nc.vector.tensor_copy(out=po_f[:], in_=po_t[:])
                nc.sync.dma_start(out=dbg_po[:, :], in_=po_f[:])
            nc.vector.reciprocal(out=rs[:], in_=po_t[:, CH : CH + 1])
            nc.vector.tensor_scalar_mul(
                out=o_sb[:, CH * h : CH * h + CH],
                in0=po_t[:, 0:CH],
                scalar1=rs[:],
            )

        for q in range(QB):
            z_slab(q, 0)
        for q in range(QB):
            z_slab(q, 1)
        for h in range(H):
            attn_head(h)
        zctx.close()

        # ---- gate, output projection ----
        if dbg:
            nc.sync.dma_start(out=dbg_osb[:, :], in_=o_sb[:])
        nc.vector.tensor_mul(out=o_sb[:], in0=o_sb[:], in1=g_sb[:])

        with tc.tile_pool(name="opsum", bufs=2, space="PSUM") as opsum:
            goT = [small.tile([128, 128], f32, tag=f"goT{d}", name=f"goT{d}") for d in range(DC)]
            for d in range(DC):
                pt = opsum.tile([128, 128], f32, tag="goTp")
                nc.tensor.transpose(pt[:], o_sb[:, d * 128 : (d + 1) * 128], ident[:])
                nc.any.tensor_copy(out=goT[d][:], in_=pt[:])
            po = opsum.tile([128, D], f32, tag="po")
            for d in range(DC):
                wo_t = wpool.tile([128, D], f32, tag="wv")
                nc.sync.dma_start(out=wo_t[:], in_=wo_d[d * 128 : (d + 1) * 128, :])
                nc.tensor.matmul(
                    po[:, 0:512], goT[d][:], wo_t[:, 0:512],
                    start=(d == 0), stop=(d == DC - 1),
                )
                nc.tensor.matmul(
                    po[:, 512:D], goT[d][:], wo_t[:, 512:D],
                    start=(d == 0), stop=(d == DC - 1),
                )
            out_sb = persist.tile([128, D], f32, tag="out_sb")
            nc.vector.tensor_copy(out=out_sb[:], in_=po[:])
        nc.sync.dma_start(out=out_d[:, :], in_=out_sb[:])
        if dbg:
            nc.sync.dma_start(out=dbg_kt[:, :], in_=KT[0][:])
            nc.sync.dma_start(out=dbg_qt[:, :], in_=QT[0][:])
            nc.sync.dma_start(out=dbg_v[:, :], in_=V[0][:])
            nc.sync.dma_start(out=dbg_g[:, :], in_=g_sb[:])
            nc.sync.dma_start(
                out=dbg_bias[:, :],
                in_=bias_g[0][:].rearrange("p j q h -> p (j q h)"),
            )


    nc.finalize()
    return nc


def kernel(**inputs):
    from concourse.bass_utils import run_bass_kernel_spmd

    if "nc" not in _CACHE:
        _CACHE["nc"] = _build_nc()
    nc = _CACHE["nc"]

    a = np.ascontiguousarray(np.asarray(inputs["a"], np.float32).reshape(N, D))
    z = np.asarray(inputs["z"], np.float32).reshape(N, N, CZ)
    mask = np.ascontiguousarray(np.asarray(inputs["mask"], np.float32).reshape(N))
    common = {
        "a": a,
        "mask": mask,
        "ln_a_w": np.ascontiguousarray(np.asarray(inputs["ln_a_w"], np.float32)),
        "ln_a_b": np.ascontiguousarray(np.asarray(inputs["ln_a_b"], np.float32)),
        "ln_z_w": np.ascontiguousarray(np.asarray(inputs["ln_z_w"], np.float32)),
        "ln_z_b": np.ascontiguousarray(np.asarray(inputs["ln_z_b"], np.float32)),
        "Wq": np.ascontiguousarray(np.asarray(inputs["Wq"], np.float32)),
        "bq": np.ascontiguousarray(np.asarray(inputs["bq"], np.float32)),
        "Wk": np.ascontiguousarray(np.asarray(inputs["Wk"], np.float32)),
        "Wv": np.ascontiguousarray(np.asarray(inputs["Wv"], np.float32)),
        "Wb": np.ascontiguousarray(np.asarray(inputs["Wb"], np.float32)),
        "Wg": np.ascontiguousarray(np.asarray(inputs["Wg"], np.float32)),
        "Wo": np.ascontiguousarray(np.asarray(inputs["Wo"], np.float32)),
    }
    in_maps = []
    for i in range(NCORES):
        m = dict(common)
        m["aq"] = np.ascontiguousarray(a[i * QB : (i + 1) * QB])
        m["zq"] = np.ascontiguousarray(
            z[i * QB : (i + 1) * QB].reshape(QB * N, CZ)
        )
        in_maps.append(m)

    res = run_bass_kernel_spmd(nc, in_maps, core_ids=list(range(NCORES)))
    out = np.concatenate([r["out"] for r in res.results], axis=0)
    return out.reshape(B, N, D).astype(np.float32)
